# revision 1
# baseline (speedup 1.0000x reference)
"""Trainium2 Bass kernel: 4-layer GPT (B=8,T=512,D=1024,H=16/KV4,FF=4096,V=32000) + LM head.

Sharding: data-parallel over batch — 8 sequences onto 8 NeuronCores, no collectives.
Device kernel keeps activations transposed ([D, T]) so every GEMM is
lhsT=W_tile (stationary), rhs=x^T (moving) with zero on-device transposes.
LayerNorm gains are folded into the following weight matrix on the host;
RoPE runs in a de-interleaved basis (host-permuted Wq/Wk columns) so it is a
32-partition block swap + 3 elementwise ops. Causal attention computes
S^T = K^T-stationary x Q with per-chunk shrinking N (skips fully-masked work),
exp without max-subtraction (scores are O(1) for this model), and gets the
softmax denominator for free from a ones-column appended to V.
"""
import os
import sys
import numpy as np

for _p in ("/opt/trn_rl_repo",):
    if _p not in sys.path:
        sys.path.insert(0, _p)

import concourse.bass as bass
import concourse.mybir as mybir
import concourse.tile as tile
import concourse.bacc as bacc
from concourse.bass_utils import run_bass_kernel_spmd

B, T, D, H, KVH, HD, L, V, FF = 8, 512, 1024, 16, 4, 64, 4, 32000, 4096
P = 128
ND = D // P          # 8 d-tiles
NT = T // P          # 4 token chunks
NFQ = (H * HD) // P  # 8 q feature tiles (head pairs)
NFK = (KVH * HD) // P  # 2 kv feature tiles
NFF = FF // P        # 32
VCH = 500
NV = V // VCH        # 64
NBIAS = NFQ + KVH + ND + NFF + ND  # 60 bias columns per layer (k bias duplicated per kv head)
F32 = mybir.dt.float32
BF16 = mybir.dt.bfloat16
AF = mybir.ActivationFunctionType

LAST_RESULTS = None  # BassKernelResults of the most recent run (for test.py)


# ---------------------------------------------------------------- device ---
def build_program():
    nc = bacc.Bacc(None, target_bir_lowering=False)

    xT_d = nc.dram_tensor("xT", [D, T], F32, kind="ExternalInput")
    wq_d = nc.dram_tensor("wq", [L, NFQ, P, ND * P], BF16, kind="ExternalInput")
    wk_d = nc.dram_tensor("wk", [L, KVH, P, ND * P], BF16, kind="ExternalInput")
    wv_d = nc.dram_tensor("wv", [L, 2, P, 4 * 256], BF16, kind="ExternalInput")
    wcp_d = nc.dram_tensor("wcp", [L, ND, P, ND * P], BF16, kind="ExternalInput")
    wfc_d = nc.dram_tensor("wfc", [L, NFF, P, ND * P], BF16, kind="ExternalInput")
    wpj_d = nc.dram_tensor("wpj", [L, ND, 4, P, 8 * P], BF16, kind="ExternalInput")
    lmw_d = nc.dram_tensor("lmw", [NV, P, ND * VCH], BF16, kind="ExternalInput")
    bias_d = nc.dram_tensor("bias", [L, P, NBIAS], F32, kind="ExternalInput")
    cc4_d = nc.dram_tensor("cc4", [P, T], F32, kind="ExternalInput")
    ss4_d = nc.dram_tensor("ss4", [P, T], F32, kind="ExternalInput")
    dmask_d = nc.dram_tensor("dmask", [P, P], F32, kind="ExternalInput")
    out_d = nc.dram_tensor("out", [T, V], F32, kind="ExternalOutput")

    with tile.TileContext(nc) as tc:
        with (
            tc.tile_pool(name="pers", bufs=1) as pers,
            tc.tile_pool(name="wsmall", bufs=3) as wsmall,
            tc.tile_pool(name="tmp", bufs=2) as tmp,
            tc.tile_pool(name="stage", bufs=4) as stage,
            tc.tile_pool(name="expp", bufs=4) as expp,
            tc.tile_pool(name="statsb", bufs=1) as statsb,
            tc.tile_pool(name="statrr", bufs=2) as statrr,
            tc.tile_pool(name="biasp", bufs=2) as biasp,
            tc.tile_pool(name="psmm", bufs=2, space="PSUM") as psmm,
            tc.tile_pool(name="psy", bufs=2, space="PSUM") as psy,
            tc.tile_pool(name="psstat", bufs=2, space="PSUM") as psstat,
            tc.tile_pool(name="psbc", bufs=2, space="PSUM") as psbc,
        ):
            # persistent tiles
            xT = pers.tile([P, ND * T], F32, tag="xT")
            lnT = pers.tile([P, ND * T], BF16, tag="lnT")
            kT = pers.tile([P, KVH * T], BF16, tag="kT")
            vA = pers.tile([P, NT * 4 * 65], BF16, tag="vA")
            yT = pers.tile([P, ND * T], BF16, tag="yT")
            cc4 = pers.tile([P, T], F32, tag="cc4")
            ss4 = pers.tile([P, T], F32, tag="ss4")
            dmask = pers.tile([P, P], F32, tag="dmask")
            onescol = pers.tile([P, 1], BF16, tag="onescol")
            onesrow = pers.tile([1, P], BF16, tag="onesrow")

            nc.sync.dma_start(cc4[:], cc4_d[:])
            nc.sync.dma_start(ss4[:], ss4_d[:])
            nc.sync.dma_start(dmask[:], dmask_d[:])
            nc.vector.memset(onescol[:], 1.0)
            nc.vector.memset(onesrow[:], 1.0)
            for c in range(NT):
                for h in range(KVH):
                    nc.vector.memset(vA[:, 260 * c + 65 * h + 64 : 260 * c + 65 * h + 65], 1.0)
            for j in range(ND):
                nc.sync.dma_start(xT[:, T * j : T * (j + 1)], xT_d[P * j : P * (j + 1), :])

            def ln_pass(src, dst):
                """dst = (src - mean)/std per column (over the partition x 8-tile D dim)."""
                sum_ps = psstat.tile([1, T], F32, tag="st")
                sq_ps = psstat.tile([1, T], F32, tag="st")
                for j in range(ND):
                    s = src[:, T * j : T * (j + 1)]
                    xb = tmp.tile([P, T], BF16, tag="lnxb")
                    sq = tmp.tile([P, T], BF16, tag="lnsq")
                    nc.vector.tensor_copy(xb[:], s)
                    nc.vector.tensor_mul(sq[:], s, s)
                    nc.tensor.matmul(sum_ps[:], onescol[:], xb[:], start=(j == 0), stop=(j == ND - 1))
                    nc.tensor.matmul(sq_ps[:], onescol[:], sq[:], start=(j == 0), stop=(j == ND - 1))
                m = statsb.tile([1, T], F32, tag="m")
                e2 = statsb.tile([1, T], F32, tag="e2")
                msq = statsb.tile([1, T], F32, tag="msq")
                sd = statsb.tile([1, T], F32, tag="sd")
                arow = statsb.tile([1, T], F32, tag="ar")
                crow = statsb.tile([1, T], F32, tag="cr")
                nc.vector.tensor_scalar_mul(m[:], sum_ps[:], 1.0 / D)
                nc.vector.tensor_scalar_mul(e2[:], sq_ps[:], 1.0 / D)
                nc.vector.tensor_mul(msq[:], m[:], m[:])
                nc.vector.tensor_sub(e2[:], e2[:], msq[:])
                nc.vector.tensor_scalar_add(e2[:], e2[:], 1e-5)
                nc.scalar.activation(sd[:], e2[:], AF.Sqrt)
                nc.vector.reciprocal(arow[:], sd[:])
                nc.vector.tensor_mul(crow[:], m[:], arow[:])
                nc.vector.tensor_scalar_mul(crow[:], crow[:], -1.0)
                arow_b = statsb.tile([1, T], BF16, tag="arb")
                crow_b = statsb.tile([1, T], BF16, tag="crb")
                nc.vector.tensor_copy(arow_b[:], arow[:])
                nc.vector.tensor_copy(crow_b[:], crow[:])
                A_ps = psbc.tile([P, T], F32, tag="bc")
                C_ps = psbc.tile([P, T], F32, tag="bc")
                nc.tensor.matmul(A_ps[:], onesrow[:], arow_b[:], start=True, stop=True)
                nc.tensor.matmul(C_ps[:], onesrow[:], crow_b[:], start=True, stop=True)
                for j in range(ND):
                    d = dst[:, T * j : T * (j + 1)]
                    nc.vector.tensor_mul(d, src[:, T * j : T * (j + 1)], A_ps[:])
                    nc.vector.tensor_add(d, d, C_ps[:])

            def rope(src, dst):
                """dst = src*cc4 + swap32(src)*ss4 (src is clobbered)."""
                sw = tmp.tile([P, T], F32, tag="swp")
                for a, b in ((0, 32), (64, 96)):
                    nc.vector.tensor_copy(sw[a : a + 32, :], src[b : b + 32, :])
                    nc.vector.tensor_copy(sw[b : b + 32, :], src[a : a + 32, :])
                nc.vector.tensor_mul(sw[:], sw[:], ss4[:])
                nc.vector.tensor_mul(src[:], src[:], cc4[:])
                nc.vector.tensor_add(dst, src[:], sw[:])

            def gemm8(w, rhs_tile, ps):
                """ps [P, T] = sum_j w[:, Pj:P(j+1)].T @ rhs_tile[:, Tj:T(j+1)]"""
                for j in range(ND):
                    nc.tensor.matmul(
                        ps[:], w[:, P * j : P * (j + 1)], rhs_tile[:, T * j : T * (j + 1)],
                        start=(j == 0), stop=(j == ND - 1),
                    )

            for l in range(L):
                btile = biasp.tile([P, NBIAS], F32, tag="bias")
                nc.sync.dma_start(btile[:], bias_d[l])
                ln_pass(xT, lnT)

                # --- K: one row-duplicated tile per kv head -> kT (roped) ---
                for i in range(KVH):
                    w = wsmall.tile([P, ND * P], BF16, tag="w")
                    nc.sync.dma_start(w[:], wk_d[l, i])
                    ps = psmm.tile([P, T], F32, tag="mm")
                    gemm8(w, lnT, ps)
                    kraw = tmp.tile([P, T], F32, tag="qraw")
                    nc.scalar.activation(kraw[:], ps[:], AF.Identity,
                                         bias=btile[:, NFQ + i : NFQ + i + 1])
                    rope(kraw, kT[:, T * i : T * (i + 1)])

                # --- V: natural layout [tok, vfeat] + ones column ---
                wv0 = wsmall.tile([P, 4 * 256], BF16, tag="w")
                wv1 = wsmall.tile([P, 4 * 256], BF16, tag="w")
                nc.sync.dma_start(wv0[:], wv_d[l, 0])
                nc.sync.dma_start(wv1[:], wv_d[l, 1])
                for c in range(NT):
                    ps = psmm.tile([P, T], F32, tag="mm")
                    for j in range(ND):
                        wvt = wv0 if j < 4 else wv1
                        nc.tensor.matmul(
                            ps[:, 0:256],
                            lnT[:, T * j + P * c : T * j + P * (c + 1)],
                            wvt[:, 256 * (j % 4) : 256 * (j % 4 + 1)],
                            start=(j == 0), stop=(j == ND - 1),
                        )
                    for h in range(KVH):
                        nc.vector.tensor_copy(vA[:, 260 * c + 65 * h : 260 * c + 65 * h + 64],
                                              ps[:, 64 * h : 64 * h + 64])

                # --- Q + attention, one head-pair tile at a time ---
                for i in range(NFQ):
                    w = wsmall.tile([P, ND * P], BF16, tag="w")
                    nc.sync.dma_start(w[:], wq_d[l, i])
                    ps = psmm.tile([P, T], F32, tag="mm")
                    gemm8(w, lnT, ps)
                    qraw = tmp.tile([P, T], F32, tag="qraw")
                    nc.scalar.activation(qraw[:], ps[:], AF.Identity,
                                         bias=btile[:, i : i + 1])
                    rq = tmp.tile([P, T], BF16, tag="rq")
                    rope(qraw, rq[:])
                    for sub in range(2):
                        h = 2 * i + sub
                        kv = h // 4
                        y_ps = psy.tile([P, T], F32, tag="y")
                        for c in range(NT):
                            N = T - P * c
                            s_ps = psmm.tile([P, T], F32, tag="mm")
                            nc.tensor.matmul(
                                s_ps[:, 0:N],
                                kT[64 * sub : 64 * sub + 64,
                                   T * kv + P * c : T * kv + P * (c + 1)],
                                rq[64 * sub : 64 * sub + 64, P * c : T],
                                start=True, stop=True,
                            )
                            nc.vector.tensor_add(s_ps[:, 0:P], s_ps[:, 0:P], dmask[:])
                            ex = expp.tile([P, T], BF16, tag="ex")
                            nc.scalar.activation(ex[:, 0:N], s_ps[:, 0:N], AF.Exp, scale=0.125)
                            nc.tensor.matmul(
                                y_ps[0:65, P * c : T],
                                vA[:, 260 * c + 65 * kv : 260 * c + 65 * kv + 65],
                                ex[:, 0:N],
                                start=(c == 0), stop=(c == NT - 1),
                            )
                        rrow = statrr.tile([1, T], F32, tag="rr")
                        rrow_b = statrr.tile([1, T], BF16, tag="rrb")
                        nc.vector.reciprocal(rrow[:], y_ps[64:65, :])
                        nc.vector.tensor_copy(rrow_b[:], rrow[:])
                        R_ps = psbc.tile([P, T], F32, tag="bc")
                        nc.tensor.matmul(R_ps[0:64, :], onesrow[0:1, 0:64], rrow_b[:],
                                         start=True, stop=True)
                        R_sb = tmp.tile([P, T], F32, tag="rsb")
                        nc.scalar.copy(R_sb[0:64, :], R_ps[0:64, :])
                        nc.vector.tensor_mul(
                            yT[64 * sub : 64 * sub + 64, T * i : T * (i + 1)],
                            y_ps[0:64, :], R_sb[0:64, :],
                        )

                # --- attention out projection + residual ---
                for fo in range(ND):
                    w = wsmall.tile([P, ND * P], BF16, tag="w")
                    nc.sync.dma_start(w[:], wcp_d[l, fo])
                    ps = psmm.tile([P, T], F32, tag="mm")
                    gemm8(w, yT, ps)
                    ct = tmp.tile([P, T], F32, tag="cptmp")
                    nc.scalar.activation(ct[:], ps[:], AF.Identity,
                                         bias=btile[:, NFQ + KVH + fo : NFQ + KVH + fo + 1])
                    x = xT[:, T * fo : T * (fo + 1)]
                    nc.vector.tensor_add(x, x, ct[:])

                # --- MLP ---
                ln_pass(xT, lnT)
                with tc.tile_pool(name="ffp", bufs=1) as ffp:
                    ffT = ffp.tile([P, NFF * T], BF16, tag="ffT")
                    for f in range(NFF):
                        w = wsmall.tile([P, ND * P], BF16, tag="w")
                        nc.sync.dma_start(w[:], wfc_d[l, f])
                        ps = psmm.tile([P, T], F32, tag="mm")
                        gemm8(w, lnT, ps)
                        bcol = NFQ + KVH + ND + f
                        nc.scalar.activation(ffT[:, T * f : T * (f + 1)], ps[:], AF.Gelu,
                                             bias=btile[:, bcol : bcol + 1])
                    for dout in range(ND):
                        ps = psmm.tile([P, T], F32, tag="mm")
                        for q in range(4):
                            w = wsmall.tile([P, 8 * P], BF16, tag="w")
                            nc.sync.dma_start(w[:], wpj_d[l, dout, q])
                            for f8 in range(8):
                                f = 8 * q + f8
                                nc.tensor.matmul(
                                    ps[:], w[:, P * f8 : P * (f8 + 1)],
                                    ffT[:, T * f : T * (f + 1)],
                                    start=(f == 0), stop=(f == NFF - 1),
                                )
                        bcol = NFQ + KVH + ND + NFF + dout
                        ct = tmp.tile([P, T], F32, tag="cptmp")
                        nc.scalar.activation(ct[:], ps[:], AF.Identity,
                                             bias=btile[:, bcol : bcol + 1])
                        x = xT[:, T * dout : T * (dout + 1)]
                        nc.vector.tensor_add(x, x, ct[:])

            # --- final LN + LM head ---
            ln_pass(xT, lnT)
            with tc.tile_pool(name="wbig", bufs=3) as wbig:
                for v in range(NV):
                    w = wbig.tile([P, ND * VCH], BF16, tag="lw")
                    nc.sync.dma_start(w[:], lmw_d[v])
                    for t in range(NT):
                        ps = psmm.tile([P, T], F32, tag="mm")
                        for j in range(ND):
                            nc.tensor.matmul(
                                ps[:, 0:VCH],
                                lnT[:, T * j + P * t : T * j + P * (t + 1)],
                                w[:, VCH * j : VCH * (j + 1)],
                                start=(j == 0), stop=(j == ND - 1),
                            )
                        st = stage.tile([P, VCH], F32, tag="lmst")
                        nc.vector.tensor_copy(st[:], ps[:, 0:VCH])
                        nc.sync.dma_start(
                            out_d[P * t : P * (t + 1), VCH * v : VCH * (v + 1)], st[:]
                        )

    nc.finalize()
    return nc


# ------------------------------------------------------------------ host ---
def _prep(inputs):
    perm = np.concatenate([np.arange(0, HD, 2), np.arange(1, HD, 2)])
    inv = 1.0 / (10000.0 ** (np.arange(0, HD, 2, dtype=np.float64) / HD))
    ang = inv[:, None] * np.arange(T, dtype=np.float64)[None, :]
    cos_t = np.cos(ang).astype(np.float32)
    sin_t = np.sin(ang).astype(np.float32)
    cc4 = np.ascontiguousarray(np.tile(cos_t, (4, 1)))
    ss4 = np.ascontiguousarray(np.concatenate([-sin_t, sin_t, -sin_t, sin_t], 0))
    r = np.arange(P)
    dmask = np.where(r[:, None] <= r[None, :], 0.0, -1e30).astype(np.float32)

    import ml_dtypes
    f32 = lambda a: np.ascontiguousarray(a, dtype=np.float32)
    bf = lambda a: np.ascontiguousarray(np.asarray(a, dtype=ml_dtypes.bfloat16))
    wq = np.empty((L, NFQ, P, ND * P), np.float32)
    wk = np.empty((L, KVH, P, ND * P), np.float32)
    wv = np.empty((L, 2, P, 4 * 256), np.float32)
    wcp = np.empty((L, ND, P, ND * P), np.float32)
    wfc = np.empty((L, NFF, P, ND * P), np.float32)
    wpj = np.empty((L, ND, 4, P, 8 * P), np.float32)
    bias = np.empty((L, P, NBIAS), np.float32)
    for l in range(L):
        Wa = inputs["ln1_g"][l][:, None] * inputs["c_attn_w"][l]
        ba = inputs["ln1_b"][l] @ inputs["c_attn_w"][l] + inputs["c_attn_b"][l]
        Wq = Wa[:, : H * HD].reshape(D, H, HD)[:, :, perm].reshape(D, H * HD)
        bq = ba[: H * HD].reshape(H, HD)[:, perm].reshape(H * HD)
        Wk = Wa[:, H * HD : H * HD + KVH * HD].reshape(D, KVH, HD)[:, :, perm].reshape(D, KVH * HD)
        bk = ba[H * HD : H * HD + KVH * HD].reshape(KVH, HD)[:, perm].reshape(KVH * HD)
        Wv = Wa[:, H * HD + KVH * HD :]
        bv = ba[H * HD + KVH * HD :]
        bv_exp = np.repeat(bv.reshape(KVH, HD), H // KVH, axis=0).reshape(H * HD)
        bcp = inputs["c_proj_b"][l] + bv_exp @ inputs["c_proj_w"][l]
        Wfc = inputs["ln2_g"][l][:, None] * inputs["fc_w"][l]
        bfc = inputs["ln2_b"][l] @ inputs["fc_w"][l] + inputs["fc_b"][l]
        Wpj, bpj = inputs["proj_w"][l], inputs["proj_b"][l]

        wq[l] = Wq.reshape(ND, P, NFQ, P).transpose(2, 1, 0, 3).reshape(NFQ, P, ND * P)
        # K: one [D, 128] block per kv head with the head's 64 columns duplicated
        # into both output-row halves, so the roped K tile is row-duplicated.
        Wk_dup = np.concatenate(
            [np.tile(Wk[:, HD * kv : HD * (kv + 1)], (1, 2)) for kv in range(KVH)], axis=1
        )  # [D, KVH*128]
        wk[l] = Wk_dup.reshape(ND, P, KVH, P).transpose(2, 1, 0, 3).reshape(KVH, P, ND * P)
        wvr = Wv.reshape(ND, P, 256)
        wv[l, 0] = wvr[0:4].transpose(1, 0, 2).reshape(P, 4 * 256)
        wv[l, 1] = wvr[4:8].transpose(1, 0, 2).reshape(P, 4 * 256)
        wcp[l] = inputs["c_proj_w"][l].reshape(ND, P, ND, P).transpose(2, 1, 0, 3).reshape(ND, P, ND * P)
        wfc[l] = Wfc.reshape(ND, P, NFF, P).transpose(2, 1, 0, 3).reshape(NFF, P, ND * P)
        wpj[l] = Wpj.reshape(4, 8, P, ND, P).transpose(3, 0, 2, 1, 4).reshape(ND, 4, P, 8 * P)
        bk_dup = np.concatenate([np.tile(bk[HD * kv : HD * (kv + 1)], 2) for kv in range(KVH)])
        bias[l] = np.concatenate(
            [bq.reshape(NFQ, P).T, bk_dup.reshape(KVH, P).T, bcp.reshape(ND, P).T,
             bfc.reshape(NFF, P).T, bpj.reshape(ND, P).T], axis=1)

    lmW = inputs["lnf_g"][:, None] * inputs["lm_w"]
    lmw = lmW.reshape(ND, P, NV, VCH).transpose(2, 1, 0, 3).reshape(NV, P, ND * VCH)
    logits_b = inputs["lnf_b"] @ inputs["lm_w"]

    common = dict(
        wq=bf(wq), wk=bf(wk), wv=bf(wv), wcp=bf(wcp), wfc=bf(wfc),
        wpj=bf(wpj), lmw=bf(lmw), bias=f32(bias), cc4=cc4, ss4=ss4, dmask=dmask,
    )
    return common, logits_b


def kernel(**inputs):
    global LAST_RESULTS
    inputs = {k: np.asarray(v) for k, v in inputs.items()}
    ids = inputs["input_ids"].astype(np.int64)
    common, logits_b = _prep(inputs)

    in_maps = []
    for b in range(B):
        xT = np.ascontiguousarray(inputs["wte"][ids[b]].T.astype(np.float32))
        in_maps.append({**common, "xT": xT})

    nc = build_program()
    trace = os.environ.get("KBENCH_TRACE", "0") == "1"
    res = run_bass_kernel_spmd(nc, in_maps, core_ids=list(range(B)), trace=trace)
    LAST_RESULTS = res

    out = np.stack([res.results[b]["out"] for b in range(B)], axis=0)
    if np.any(logits_b != 0.0):
        out = out + logits_b[None, None, :].astype(np.float32)
    return out


if __name__ == "__main__":
    import reference
    inp = {k: np.asarray(v) for k, v in reference.setup_inputs().items()}
    got = kernel(**inp)
    exp = np.asarray(reference.reference(**reference.setup_inputs()))
    rel = np.linalg.norm(got - exp) / np.linalg.norm(exp)
    print("Relative error:", rel)



# revision 10
# speedup vs baseline: 1.2046x; 1.2046x over previous
"""Trainium2 Bass kernel: 4-layer GPT (B=8,T=512,D=1024,H=16/KV4,FF=4096,V=32000) + LM head.

Sharding: data-parallel over batch - 8 sequences onto 8 NeuronCores, no collectives.
Activations stay transposed ([D, T]); every GEMM is lhsT=W_tile stationary.

Engine plan (vs. the previous version): weight loads are bundled into ~8KB/partition
DMAs; LN statistics are computed by f32r/bf16 matmuls that chase the residual adds
(no DVE prep), with rstd = exp(-0.5*ln(D*ssq - sum^2) + ln D) so the Ln/Exp table
stays resident through attention; rope's partition swap is a single permutation
matmul on the PE instead of four DVE copies; GQA is laid out pair-aligned so K needs
no row duplication; causal masking is a cheap bf16 multiply after exp; softmax
denominators are batched per head-pair (one reciprocal + one broadcast matmul);
residual adds/squares/copies run on the idle Pool engine; the LM head writes bf16.
"""
import os
import sys
import numpy as np

for _p in ("/opt/trn_rl_repo",):
    if _p not in sys.path:
        sys.path.insert(0, _p)

import concourse.bass as bass
import concourse.mybir as mybir
import concourse.tile as tile
import concourse.bacc as bacc
from concourse.bass_utils import run_bass_kernel_spmd

B, T, D, H, KVH, HD, L, V, FF = 8, 512, 1024, 16, 4, 64, 4, 32000, 4096
P = 128
ND = D // P            # 8 d-tiles
NT = T // P            # 4 token chunks
NP = 8                 # 8 q head-pair tiles
NKT = 2                # 2 k feature tiles
NFF = FF // P          # 32
VCH = 500
NV = V // VCH          # 64
NB = NP + NKT + ND + NFF + ND  # 58 bias columns
F32 = mybir.dt.float32
BF16 = mybir.dt.bfloat16
AF = mybir.ActivationFunctionType

# pair p holds head PAIRS[p][0] in rows 0:64 (kv head even) and PAIRS[p][1]
# in rows 64:128 (kv head odd), so every S matmul is partition-aligned with kT.
PAIRS = [(0, 4), (1, 5), (2, 6), (3, 7), (8, 12), (9, 13), (10, 14), (11, 15)]

LAST_RESULTS = None  # BassKernelResults of the most recent run (for test.py)


# ---------------------------------------------------------------- device ---
def build_program():
    nc = bacc.Bacc(None, target_bir_lowering=False)

    xT_d = nc.dram_tensor("xT", [D, T], F32, kind="ExternalInput")
    wq_d = nc.dram_tensor("wq", [L, 2, P, 4 * ND * P], BF16, kind="ExternalInput")
    wk_d = nc.dram_tensor("wk", [L, P, NKT * ND * P], BF16, kind="ExternalInput")
    wv_d = nc.dram_tensor("wv", [L, P, 2048], BF16, kind="ExternalInput")
    wcp_d = nc.dram_tensor("wcp", [L, 2, P, 4 * ND * P], BF16, kind="ExternalInput")
    wfc_d = nc.dram_tensor("wfc", [L, 8, P, 4 * ND * P], BF16, kind="ExternalInput")
    wpj_d = nc.dram_tensor("wpj", [L, ND, P, NFF * P], BF16, kind="ExternalInput")
    lmw_d = nc.dram_tensor("lmw", [NV, P, ND * VCH], BF16, kind="ExternalInput")
    bias_d = nc.dram_tensor("bias", [L, P, NB], F32, kind="ExternalInput")
    cc4_d = nc.dram_tensor("cc4", [P, T], F32, kind="ExternalInput")
    ss4_d = nc.dram_tensor("ss4", [P, T], F32, kind="ExternalInput")
    perm_d = nc.dram_tensor("permm", [P, P], F32, kind="ExternalInput")
    mask_d = nc.dram_tensor("maskb", [P, P], BF16, kind="ExternalInput")
    out_d = nc.dram_tensor("out", [T, V], BF16, kind="ExternalOutput")

    with tile.TileContext(nc) as tc:
        with (
            tc.tile_pool(name="pers", bufs=1) as pers,
            tc.tile_pool(name="wpool", bufs=1) as wpool,
            tc.tile_pool(name="tf", bufs=1) as tf,
            tc.tile_pool(name="rows", bufs=1) as rows,
            tc.tile_pool(name="pso", bufs=1, space="PSUM") as pso,
        ):
            # ------------------------------------------------ persistents ---
            xT = pers.tile([P, ND * T], F32, tag="xT")
            lnT = pers.tile([P, ND * T], BF16, tag="lnT")
            rqA = pers.tile([P, NP * T], BF16, tag="rqA")
            kT = pers.tile([P, NKT * T], BF16, tag="kT")
            vA = pers.tile([P, NT * 4 * 65], BF16, tag="vA")
            yT = pers.tile([P, ND * T], BF16, tag="yT")
            ffT = pers.tile([P, NFF * T], BF16, tag="ffT")
            cc4 = pers.tile([P, T], F32, tag="cc4")
            cc4b = pers.tile([P, T], BF16, tag="cc4b")
            ss4 = pers.tile([P, T], F32, tag="ss4")
            permb = pers.tile([P, P], BF16, tag="permb")
            maskb = pers.tile([P, P], BF16, tag="maskb")
            onescol = pers.tile([P, 1], BF16, tag="onescol")
            dcol = pers.tile([P, 1], BF16, tag="dcol")
            onesrow = pers.tile([1, P], BF16, tag="onesrow")
            negDrow = pers.tile([1, P], BF16, tag="negDrow")
            epsr = pers.tile([1, 1], F32, tag="epsr")
            lnDr = pers.tile([1, 1], F32, tag="lnDr")
            sel2 = pers.tile([33, P], BF16, tag="sel2")
            den2 = pers.tile([33, T], F32, tag="den2")

            nc.sync.dma_start(cc4[:], cc4_d[:])
            nc.vector.tensor_copy(cc4b[:], cc4[:])
            nc.sync.dma_start(ss4[:], ss4_d[:])
            permf = pers.tile([P, P], F32, tag="permf")
            nc.sync.dma_start(permf[:], perm_d[:])
            nc.vector.tensor_copy(permb[:], permf[:])
            nc.sync.dma_start(maskb[:], mask_d[:])
            nc.vector.memset(onescol[:], 1.0)
            nc.vector.memset(dcol[:], float(D))
            nc.vector.memset(onesrow[:], 1.0)
            nc.vector.memset(negDrow[:], -1.0 / D)
            nc.vector.memset(epsr[:], float(D) * D * 1e-5)
            nc.vector.memset(lnDr[:], float(np.log(D)))
            nc.vector.memset(sel2[:], 0.0)
            nc.vector.memset(sel2[0:1, 0:64], 1.0)
            nc.vector.memset(sel2[32:33, 64:128], 1.0)
            nc.vector.memset(den2[:], 1.0)
            for c in range(NT):
                for kv in range(KVH):
                    nc.vector.memset(vA[:, 260 * c + 65 * kv + 64 : 260 * c + 65 * kv + 65], 1.0)

            # ----------------------------------------------------- helpers ---
            def gemm8(w, off, rhs_tile, ps, n=ND):
                """ps[P,T] = sum_j w[:, off+P*j : off+P*(j+1)].T @ rhs_tile tile j"""
                for j in range(n):
                    nc.tensor.matmul(
                        ps[:], w[:, off + P * j : off + P * (j + 1)],
                        rhs_tile[:, T * j : T * (j + 1)],
                        start=(j == 0), stop=(j == n - 1),
                    )

            def chase_stats(stat, j, first, last):
                """Accumulate per-token sum and D*sum(x^2) for d-tile j."""
                xb = tf.tile([P, T], BF16, tag="xb", bufs=2)
                nc.vector.tensor_copy(xb[:], xT[:, T * j : T * (j + 1)])
                sq = tf.tile([P, T], BF16, tag="sq", bufs=2)
                nc.vector.tensor_mul(sq[:], xb[:], xb[:])
                nc.tensor.matmul(stat[0:1, :], onescol[:], xb[:], start=first, stop=last)
                nc.tensor.matmul(stat[32:33, :], dcol[:], sq[:], start=first, stop=last)

            def row_chain(stat):
                """stat -> (A_ps, C_ps): lnT_j = xT_j*A + C normalizes per token."""
                trow = rows.tile([1, T], F32, tag="rowf", bufs=1)
                nc.scalar.activation(trow[:], stat[0:1, :], AF.Square)
                vrow = rows.tile([1, T], F32, tag="rowf2", bufs=1)
                nc.vector.tensor_sub(vrow[:], stat[32:33, :], trow[:])
                lnr = rows.tile([1, T], F32, tag="rowg", bufs=1)
                nc.scalar.activation(lnr[:], vrow[:], AF.Ln, bias=epsr[:])
                arow_b = rows.tile([1, T], BF16, tag="rowb", bufs=1)
                nc.scalar.activation(arow_b[:], lnr[:], AF.Exp, scale=-0.5, bias=lnDr[:])
                srow_b = rows.tile([1, T], BF16, tag="rowc", bufs=1)
                nc.vector.tensor_copy(srow_b[:], stat[0:1, :])
                crow_b = rows.tile([1, T], BF16, tag="rowd", bufs=1)
                nc.vector.tensor_mul(crow_b[:], srow_b[:], arow_b[:])
                A_ps = pso.tile([P, T], F32, tag="bc", bufs=2)
                nc.tensor.matmul(A_ps[:], onesrow[:], arow_b[:], start=True, stop=True)
                C_ps = pso.tile([P, T], F32, tag="bc", bufs=2)
                nc.tensor.matmul(C_ps[:], negDrow[:], crow_b[:], start=True, stop=True)
                return A_ps, C_ps

            def ln_apply(A_ps, C_ps):
                # Pool cannot read PSUM: DVE does the muls (PSUM A), Pool the
                # adds against an SBUF copy of C.
                C_sb = tf.tile([P, T], F32, tag="csb", bufs=2)
                nc.scalar.activation(C_sb[:], C_ps[:], AF.Identity)
                for j in range(ND):
                    tmp = tf.tile([P, T], F32, tag="tf32", bufs=4)
                    nc.vector.tensor_mul(tmp[:], xT[:, T * j : T * (j + 1)], A_ps[:])
                    nc.gpsimd.tensor_add(lnT[:, T * j : T * (j + 1)], tmp[:], C_sb[:])

            def rope_tile(w, off, dst, bias_col, psp):
                """dst[P,T](bf16) = rope(W.T @ lnT + b); swap via permutation matmul."""
                ps = psp.tile([P, T], F32, tag="ps", bufs=3)
                gemm8(w, off, lnT, ps)
                kb = tf.tile([P, T], BF16, tag="kb", bufs=2)
                nc.scalar.activation(kb[:], ps[:], AF.Identity, bias=bias_col)
                t1 = tf.tile([P, T], BF16, tag="t1", bufs=2)
                nc.vector.tensor_mul(t1[:], kb[:], cc4b[:])
                sw = psp.tile([P, T], F32, tag="sw", bufs=2)
                nc.tensor.matmul(sw[:], permb[:], kb[:], start=True, stop=True)
                t2 = tf.tile([P, T], BF16, tag="t2", bufs=2)
                nc.vector.tensor_mul(t2[:], sw[:], ss4[:])
                nc.vector.tensor_add(dst, t1[:], t2[:])

            # ------------------------------------------------ initial load ---
            btile0 = wpool.tile([P, NB], F32, tag="bias", bufs=2)
            nc.sync.dma_start(btile0[:], bias_d[0])
            for j in range(ND):
                nc.sync.dma_start(xT[:, T * j : T * (j + 1)], xT_d[P * j : P * (j + 1), :])
            stat = pso.tile([64, T], F32, tag="stat", bufs=1)
            for j in range(ND):
                chase_stats(stat, j, j == 0, j == ND - 1)

            # ---------------------------------------------------- layers ---
            for l in range(L):
                btile = btile0 if l == 0 else wpool.tile([P, NB], F32, tag="bias", bufs=2)
                if l > 0:
                    nc.sync.dma_start(btile[:], bias_d[l])

                A_ps, C_ps = row_chain(stat)
                ln_apply(A_ps, C_ps)

                # --- QKV + rope ---
                wkt = wpool.tile([P, NKT * ND * P], BF16, tag="wk", bufs=1)
                nc.sync.dma_start(wkt[:], wk_d[l])
                wvt = wpool.tile([P, 2048], BF16, tag="wv", bufs=1)
                nc.sync.dma_start(wvt[:], wv_d[l])

                with tc.tile_pool(name="ps_qkv", bufs=1, space="PSUM") as psp:
                    for i in range(NKT):
                        rope_tile(wkt, ND * P * i, kT[:, T * i : T * (i + 1)],
                                  btile[:, NP + i : NP + i + 1], psp)
                    for c in range(NT):
                        psv = psp.tile([P, T], F32, tag="ps", bufs=3)
                        for j in range(ND):
                            nc.tensor.matmul(
                                psv[:, 0:256],
                                lnT[:, T * j + P * c : T * j + P * (c + 1)],
                                wvt[:, 256 * j : 256 * (j + 1)],
                                start=(j == 0), stop=(j == ND - 1),
                            )
                        for kv in range(KVH):
                            nc.vector.tensor_copy(
                                vA[:, 260 * c + 65 * kv : 260 * c + 65 * kv + 64],
                                psv[:, 64 * kv : 64 * kv + 64])
                    wq2 = None
                    for p in range(NP):
                        if p % 2 == 0:
                            wq2 = wpool.tile([P, 2 * ND * P], BF16, tag="wq", bufs=2)
                            h2 = 2 * ND * P * ((p // 2) % 2)
                            nc.sync.dma_start(wq2[:], wq_d[l, p // 4][:, h2 : h2 + 2 * ND * P])
                        rope_tile(wq2, ND * P * (p % 2), rqA[:, T * p : T * (p + 1)],
                                  btile[:, p : p + 1], psp)

                # --- attention ---
                with tc.tile_pool(name="ps_att", bufs=1, space="PSUM") as psa:
                    for p in range(NP):
                        ys = []
                        for hh, h in enumerate(PAIRS[p]):
                            base = 64 * hh
                            kv = h // 4
                            kt = kv // 2
                            kb_ = 64 * (kv % 2)
                            rq = rqA[:, T * p : T * (p + 1)]

                            def kcol(c):
                                return kT[kb_ : kb_ + 64, T * kt + P * c : T * kt + P * (c + 1)]

                            def vcol(c):
                                return vA[:, 260 * c + 65 * kv : 260 * c + 65 * kv + 65]

                            ex = tf.tile([P, 1280], BF16, tag="ex", bufs=3)
                            y = psa.tile([65, T], F32, tag="y", bufs=2)
                            # chunk 0 (full width); ex layout: c1c3 [0:512],
                            # c0 [512:1024], c2 [1024:1280]
                            s_b = psa.tile([P, T], F32, tag="s", bufs=3)
                            nc.tensor.matmul(s_b[:, 0:T], kcol(0), rq[base : base + 64, 0:T],
                                             start=True, stop=True)
                            nc.scalar.activation(ex[:, 512:1024], s_b[:, 0:T], AF.Exp, scale=0.125)
                            nc.gpsimd.tensor_mul(ex[:, 512:640], ex[:, 512:640], maskb[:])
                            nc.tensor.matmul(y[0:65, 0:T], vcol(0), ex[:, 512:1024],
                                             start=True, stop=False)
                            # chunks 1+3 packed in one bank
                            s_a = psa.tile([P, T], F32, tag="s", bufs=3)
                            nc.tensor.matmul(s_a[:, 0:384], kcol(1), rq[base : base + 64, P:T],
                                             start=True, stop=True)
                            nc.tensor.matmul(s_a[:, 384:512], kcol(3), rq[base : base + 64, 3 * P : T],
                                             start=True, stop=True)
                            nc.scalar.activation(ex[:, 0:512], s_a[:, 0:512], AF.Exp, scale=0.125)
                            nc.tensor.matmul(y[0:65, 2 * P : T], vcol(1), ex[:, 128:384],
                                             start=False, stop=False)
                            nc.gpsimd.tensor_mul(ex[:, 0:128], ex[:, 0:128], maskb[:])
                            nc.tensor.matmul(y[0:65, P : 2 * P], vcol(1), ex[:, 0:128],
                                             start=False, stop=False)
                            nc.gpsimd.tensor_mul(ex[:, 384:512], ex[:, 384:512], maskb[:])
                            nc.tensor.matmul(y[0:65, 3 * P : T], vcol(3), ex[:, 384:512],
                                             start=False, stop=False)
                            # chunk 2
                            s_c = psa.tile([P, T], F32, tag="s", bufs=3)
                            nc.tensor.matmul(s_c[:, 0:256], kcol(2), rq[base : base + 64, 2 * P : T],
                                             start=True, stop=True)
                            nc.scalar.activation(ex[:, 1024:1280], s_c[:, 0:256], AF.Exp, scale=0.125)
                            nc.tensor.matmul(y[0:65, 3 * P : T], vcol(2), ex[:, 1152:1280],
                                             start=False, stop=False)
                            nc.gpsimd.tensor_mul(ex[:, 1024:1152], ex[:, 1024:1152], maskb[:])
                            nc.tensor.matmul(y[0:65, 2 * P : 3 * P], vcol(2), ex[:, 1024:1152],
                                             start=False, stop=True)
                            nc.vector.tensor_copy(den2[32 * hh : 32 * hh + 1, :], y[64:65, :])
                            ys.append(y)
                        # pair tail: batched denominators
                        rden = rows.tile([33, T], F32, tag="rden", bufs=2)
                        nc.vector.reciprocal(rden[0:33, :], den2[0:33, :])
                        rden_b = rows.tile([33, T], BF16, tag="rdenb", bufs=2)
                        nc.vector.tensor_copy(rden_b[0:33, :], rden[0:33, :])
                        R_ps = pso.tile([P, T], F32, tag="bc", bufs=2)
                        nc.tensor.matmul(R_ps[:], sel2[0:33, :], rden_b[0:33, :],
                                         start=True, stop=True)
                        R_sb = tf.tile([P, T], F32, tag="rsb", bufs=2)
                        nc.vector.tensor_copy(R_sb[:], R_ps[:])
                        nc.vector.tensor_mul(yT[0:64, T * p : T * (p + 1)],
                                             ys[0][0:64, :], R_sb[0:64, :])
                        nc.vector.tensor_mul(yT[64:128, T * p : T * (p + 1)],
                                             ys[1][0:64, :], R_sb[64:128, :])

                # --- attention out projection + residual + ln2 stats chase ---
                stat = pso.tile([64, T], F32, tag="stat", bufs=1)
                with tc.tile_pool(name="ps_cp", bufs=1, space="PSUM") as psc:
                    wcp2 = None
                    for dout in range(ND):
                        if dout % 2 == 0:
                            wcp2 = wpool.tile([P, 2 * ND * P], BF16, tag="wcp", bufs=2)
                            h2 = 2 * ND * P * ((dout // 2) % 2)
                            nc.sync.dma_start(wcp2[:], wcp_d[l, dout // 4][:, h2 : h2 + 2 * ND * P])
                        ps = psc.tile([P, T], F32, tag="ps", bufs=4)
                        gemm8(wcp2, ND * P * (dout % 2), yT, ps)
                        ct = tf.tile([P, T], F32, tag="tf32", bufs=4)
                        nc.scalar.activation(ct[:], ps[:], AF.Identity,
                                             bias=btile[:, NP + NKT + dout : NP + NKT + dout + 1])
                        x = xT[:, T * dout : T * (dout + 1)]
                        nc.gpsimd.tensor_add(x, x, ct[:])
                        chase_stats(stat, dout, dout == 0, dout == ND - 1)

                A_ps, C_ps = row_chain(stat)
                ln_apply(A_ps, C_ps)

                # --- MLP ---
                with tc.tile_pool(name="ps_mlp", bufs=1, space="PSUM") as psm:
                    wfc2 = None
                    for f in range(NFF):
                        if f % 2 == 0:
                            wfc2 = wpool.tile([P, 2 * ND * P], BF16, tag="wfc", bufs=2)
                            h2 = 2 * ND * P * ((f // 2) % 2)
                            nc.sync.dma_start(wfc2[:], wfc_d[l, f // 4][:, h2 : h2 + 2 * ND * P])
                        ps = psm.tile([P, T], F32, tag="ps", bufs=5)
                        gemm8(wfc2, ND * P * (f % 2), lnT, ps)
                        bcol = NP + NKT + ND + f
                        nc.scalar.activation(ffT[:, T * f : T * (f + 1)], ps[:], AF.Gelu,
                                             bias=btile[:, bcol : bcol + 1])
                    stat = pso.tile([64, T], F32, tag="stat", bufs=1)
                    for dout in range(ND):
                        wpjA = wpool.tile([P, NFF * P // 2], BF16, tag="wpj", bufs=3)
                        nc.sync.dma_start(wpjA[:], wpj_d[l, dout][:, 0 : NFF * P // 2])
                        wpjB = wpool.tile([P, NFF * P // 2], BF16, tag="wpj", bufs=3)
                        nc.sync.dma_start(wpjB[:], wpj_d[l, dout][:, NFF * P // 2 : NFF * P])
                        ps = psm.tile([P, T], F32, tag="ps", bufs=5)
                        for f in range(NFF):
                            wt, fo = (wpjA, f) if f < 16 else (wpjB, f - 16)
                            nc.tensor.matmul(
                                ps[:], wt[:, P * fo : P * (fo + 1)], ffT[:, T * f : T * (f + 1)],
                                start=(f == 0), stop=(f == NFF - 1),
                            )
                        bcol = NP + NKT + ND + NFF + dout
                        ct = tf.tile([P, T], F32, tag="tf32", bufs=4)
                        nc.scalar.activation(ct[:], ps[:], AF.Identity,
                                             bias=btile[:, bcol : bcol + 1])
                        x = xT[:, T * dout : T * (dout + 1)]
                        nc.gpsimd.tensor_add(x, x, ct[:])
                        chase_stats(stat, dout, dout == 0, dout == ND - 1)

            # --- final LN + LM head ---
            A_ps, C_ps = row_chain(stat)
            ln_apply(A_ps, C_ps)
            with tc.tile_pool(name="ps_lm", bufs=1, space="PSUM") as psl:
                for v in range(NV):
                    lwA = wpool.tile([P, 4 * VCH], BF16, tag="lmw", bufs=3)
                    nc.sync.dma_start(lwA[:], lmw_d[v][:, 0 : 4 * VCH])
                    lwB = wpool.tile([P, 4 * VCH], BF16, tag="lmw", bufs=3)
                    nc.sync.dma_start(lwB[:], lmw_d[v][:, 4 * VCH : ND * VCH])
                    for t in range(NT):
                        ps = psl.tile([P, T], F32, tag="ps", bufs=4)
                        for j in range(ND):
                            wt, jo = (lwA, j) if j < 4 else (lwB, j - 4)
                            nc.tensor.matmul(
                                ps[:, 0:VCH],
                                lnT[:, T * j + P * t : T * j + P * (t + 1)],
                                wt[:, VCH * jo : VCH * (jo + 1)],
                                start=(j == 0), stop=(j == ND - 1),
                            )
                        st = tf.tile([P, VCH], BF16, tag="st", bufs=3)
                        nc.scalar.activation(st[:], ps[:, 0:VCH], AF.Identity)
                        nc.sync.dma_start(
                            out_d[P * t : P * (t + 1), VCH * v : VCH * (v + 1)], st[:]
                        )

    nc.finalize()
    return nc


# ------------------------------------------------------------------ host ---
def _prep(inputs):
    perm64 = np.concatenate([np.arange(0, HD, 2), np.arange(1, HD, 2)])
    inv = 1.0 / (10000.0 ** (np.arange(0, HD, 2, dtype=np.float64) / HD))
    ang = inv[:, None] * np.arange(T, dtype=np.float64)[None, :]
    cos_t = np.cos(ang).astype(np.float32)
    sin_t = np.sin(ang).astype(np.float32)
    cc4 = np.ascontiguousarray(np.tile(cos_t, (4, 1)))
    ss4 = np.ascontiguousarray(np.concatenate([-sin_t, sin_t, -sin_t, sin_t], 0))

    sigma = np.concatenate([np.arange(32, 64), np.arange(0, 32),
                            np.arange(96, 128), np.arange(64, 96)])
    permm = np.zeros((P, P), np.float32)
    for m in range(P):
        permm[sigma[m], m] = 1.0
    r = np.arange(P)
    maskb = (r[:, None] <= r[None, :]).astype(np.float32)

    import ml_dtypes
    f32 = lambda a: np.ascontiguousarray(a, dtype=np.float32)
    bf = lambda a: np.ascontiguousarray(np.asarray(a, dtype=ml_dtypes.bfloat16))

    def tile8(Wcols):  # [D, 128] -> [P, ND*P] lhsT tile (cols j-major)
        return Wcols.reshape(ND, P, P).transpose(1, 0, 2).reshape(P, ND * P)

    wq = np.empty((L, 2, P, 4 * ND * P), np.float32)
    wk = np.empty((L, P, NKT * ND * P), np.float32)
    wv = np.empty((L, P, 2048), np.float32)
    wcp = np.empty((L, 2, P, 4 * ND * P), np.float32)
    wfc = np.empty((L, 8, P, 4 * ND * P), np.float32)
    wpj = np.empty((L, ND, P, NFF * P), np.float32)
    bias = np.empty((L, P, NB), np.float32)

    head_order = [h for pr in PAIRS for h in pr]
    RP = np.concatenate([np.arange(HD * h, HD * (h + 1)) for h in head_order])

    for l in range(L):
        Wa = inputs["ln1_g"][l][:, None] * inputs["c_attn_w"][l]
        ba = inputs["ln1_b"][l] @ inputs["c_attn_w"][l] + inputs["c_attn_b"][l]
        Wq = Wa[:, : H * HD]
        bq = ba[: H * HD]
        Wk = Wa[:, H * HD : H * HD + KVH * HD]
        bk = ba[H * HD : H * HD + KVH * HD]
        Wv = Wa[:, H * HD + KVH * HD :]
        bv = ba[H * HD + KVH * HD :]

        for p, (a, b) in enumerate(PAIRS):
            cols = np.concatenate([
                Wq[:, HD * a : HD * (a + 1)][:, perm64],
                Wq[:, HD * b : HD * (b + 1)][:, perm64]], axis=1)
            wq[l, p // 4, :, ND * P * (p % 4) : ND * P * (p % 4 + 1)] = tile8(cols)
            bias[l, :, p] = np.concatenate([
                bq[HD * a : HD * (a + 1)][perm64], bq[HD * b : HD * (b + 1)][perm64]])
        for i in range(NKT):
            cols = np.concatenate([
                Wk[:, HD * 2 * i : HD * (2 * i + 1)][:, perm64],
                Wk[:, HD * (2 * i + 1) : HD * (2 * i + 2)][:, perm64]], axis=1)
            wk[l, :, ND * P * i : ND * P * (i + 1)] = tile8(cols)
            bias[l, :, NP + i] = np.concatenate([
                bk[HD * 2 * i : HD * (2 * i + 1)][perm64],
                bk[HD * (2 * i + 1) : HD * (2 * i + 2)][perm64]])
        wv[l] = Wv.reshape(ND, P, 256).transpose(1, 0, 2).reshape(P, 2048)

        # v-bias folds through softmax (rows sum to 1) into the c_proj bias
        bv_exp = np.repeat(bv.reshape(KVH, HD), H // KVH, axis=0).reshape(H * HD)
        bcp = inputs["c_proj_b"][l] + bv_exp @ inputs["c_proj_w"][l]
        Wcp_r = inputs["c_proj_w"][l][RP, :]
        for dout in range(ND):
            wcp[l, dout // 4, :, ND * P * (dout % 4) : ND * P * (dout % 4 + 1)] = \
                tile8(Wcp_r[:, P * dout : P * (dout + 1)])
            bias[l, :, NP + NKT + dout] = bcp[P * dout : P * (dout + 1)]

        Wfc = inputs["ln2_g"][l][:, None] * inputs["fc_w"][l]
        bfc = inputs["ln2_b"][l] @ inputs["fc_w"][l] + inputs["fc_b"][l]
        for f in range(NFF):
            wfc[l, f // 4, :, ND * P * (f % 4) : ND * P * (f % 4 + 1)] = \
                tile8(Wfc[:, P * f : P * (f + 1)])
            bias[l, :, NP + NKT + ND + f] = bfc[P * f : P * (f + 1)]

        Wpj = inputs["proj_w"][l]
        wpj[l] = Wpj.reshape(NFF, P, ND, P).transpose(2, 1, 0, 3).reshape(ND, P, NFF * P)
        bias[l, :, NP + NKT + ND + NFF:] = inputs["proj_b"][l].reshape(ND, P).T

    lmW = inputs["lnf_g"][:, None] * inputs["lm_w"]
    lmw = lmW.reshape(ND, P, NV, VCH).transpose(2, 1, 0, 3).reshape(NV, P, ND * VCH)
    logits_b = inputs["lnf_b"] @ inputs["lm_w"]

    common = dict(
        wq=bf(wq), wk=bf(wk), wv=bf(wv), wcp=bf(wcp), wfc=bf(wfc), wpj=bf(wpj),
        lmw=bf(lmw), bias=f32(bias), cc4=cc4, ss4=ss4, permm=permm, maskb=bf(maskb),
    )
    return common, logits_b


def kernel(**inputs):
    global LAST_RESULTS
    inputs = {k: np.asarray(v) for k, v in inputs.items()}
    ids = inputs["input_ids"].astype(np.int64)
    common, logits_b = _prep(inputs)

    in_maps = []
    for b in range(B):
        xT = np.ascontiguousarray(inputs["wte"][ids[b]].T.astype(np.float32))
        in_maps.append({**common, "xT": xT})

    nc = build_program()
    trace = os.environ.get("KBENCH_TRACE", "0") == "1"
    res = run_bass_kernel_spmd(nc, in_maps, core_ids=list(range(B)), trace=trace)
    LAST_RESULTS = res

    out = np.stack([np.asarray(res.results[b]["out"]).astype(np.float32) for b in range(B)], axis=0)
    if np.any(logits_b != 0.0):
        out = out + logits_b[None, None, :].astype(np.float32)
    return out


if __name__ == "__main__":
    import reference
    inp = {k: np.asarray(v) for k, v in reference.setup_inputs().items()}
    got = kernel(**inp)
    exp = np.asarray(reference.reference(**reference.setup_inputs()))
    rel = np.linalg.norm(got - exp) / np.linalg.norm(exp)
    print("Relative error:", rel)
